# revision 21
# baseline (speedup 1.0000x reference)
"""H2GCN forward on 8 Trainium2 NeuronCores.

out = concat([h0, A1@h0, A2@h0], 1) @ W_out + b_out,  h0 = x @ W1

Data-parallel over destination nodes (1250 rows/core, padded to 1280).
Per core: h0 = x@W1 in bf16 (row-tile pipelined with the x DMA), h0
quantized to fp8-e4m3 and AllGathered in two chunks, SpMM as dense
fp8 DoubleRow matmuls with the adjacency blocks as the MOVING operand
and h0 tiles stationary -- so 256 sources contract per instruction and
the output lands feature-major (no transpose phase for h1/h2).  A1 is
pre-scaled by 16 and A2 by 32 (undone in W_out rows) to keep edge
weights in fp8's normal range.  Final GEMM in bf16 with bias as a K=1
matmul.
"""
import sys
import types

for _p in ("/opt/trn_rl_repo", "/root/.axon_site", "/root/.axon_site/_ro/trn_rl_repo",
           "/root/.axon_site/_ro/pypackages"):
    if _p not in sys.path:
        sys.path.append(_p)

import numpy as np
import ml_dtypes
import concourse.bass as bass
import concourse.bacc as bacc
import concourse.mybir as mybir
import concourse.tile as tile
from concourse import bass_utils

N, IN_C, HID, OUT_C = 10000, 2048, 256, 256
NCORES = 8
ROWS = N // NCORES          # 1250
PROWS = 1280                # padded (10 x 128)
NT = PROWS // 128           # 10 row tiles per core
KT = IN_C // 128            # 16 k tiles
ST = NCORES * NT            # 80 source tiles in the padded gather space
NG = ST // 2                # 40 source super-tiles (DoubleRow pairs)
HT0 = 6                     # row tiles in AllGather chunk 0 (rest in chunk 1)
G0 = HT0 // 2               # local super-tiles fully inside chunk 0

f32 = mybir.dt.float32
bf16 = mybir.dt.bfloat16
f8 = mybir.dt.float8e4
bfnp = ml_dtypes.bfloat16
f8np = ml_dtypes.float8_e4m3fn

# blob_a (bf16): W1 k-tiles then x row-tile-major k-tiles
OW1, OX = 0, KT * HID
BLOBA = KT * HID + NT * KT * 128
# blob_b (bf16): Wout k-tiles, bias (row 0), ones (row 0), identity
OWO, OB, OO, OI = 0, 6 * OUT_C, 6 * OUT_C + OUT_C, 6 * OUT_C + OUT_C + 128
BLOBB = OI + 128

# spmm processing order: super-tiles whose sources are in AG chunk 0 first
G_ORDER = [r * (NT // 2) + j for r in range(NCORES) for j in range(G0)] + \
          [r * (NT // 2) + j for r in range(NCORES) for j in range(G0, NT // 2)]

LAST_EXEC_NS = None
LAST_RESULTS = None


def _install_trace_shim():
    try:
        import antenv.axon_hooks  # noqa: F401
        return
    except ImportError:
        pass
    try:
        import antenv
        from trn_agent_boot.trn_boot import _ntff_profile_via_ctypes
        hook = _ntff_profile_via_ctypes("/opt/axon/libaxon_pjrt.so")
        mod = types.ModuleType("antenv.axon_hooks")
        mod.get_axon_ntff_profile_hook = lambda: hook
        mod.set_axon_ntff_profile_hook = lambda h: None
        sys.modules["antenv.axon_hooks"] = mod
        antenv.axon_hooks = mod
    except Exception:
        pass


def _pack_adj(rows, cols, vals, core, scale):
    """fp8 dense A^T for this core's dest shard, laid out
    [128 src-part, NG super, 2 ktile, PROWS dest] (flattened free dim)."""
    lo, hi = core * ROWS, (core + 1) * ROWS
    m = (rows >= lo) & (rows < hi)
    r, c, v = rows[m] - lo, cols[m], vals[m] * scale
    A = np.zeros((NCORES * PROWS, PROWS), np.float32)
    src = (c // ROWS) * PROWS + (c % ROWS)
    np.add.at(A, (src, r), v)
    return np.ascontiguousarray(
        A.reshape(NG, 2, 128, PROWS).transpose(2, 0, 1, 3)
        .reshape(128, NG * 2 * PROWS)).astype(f8np)


def _build():
    nc = bacc.Bacc("TRN2", target_bir_lowering=False, debug=False,
                   num_devices=8)
    blob_a = nc.dram_tensor("blob_a", [128, BLOBA], bf16, kind="ExternalInput")
    blob_b = nc.dram_tensor("blob_b", [128, BLOBB], bf16, kind="ExternalInput")
    A_d = nc.dram_tensor("A_d", [128, 2 * NG * 2 * PROWS], f8,
                         kind="ExternalInput")
    out = nc.dram_tensor("out", [ROWS, OUT_C], f32, kind="ExternalOutput")

    DR = mybir.MatmulPerfMode.DoubleRow

    with tile.TileContext(nc) as tc:
        with tc.tile_pool(name="keep", bufs=1) as keep, \
             tc.tile_pool(name="dram", bufs=1, space="DRAM") as dram, \
             tc.tile_pool(name="pmm", bufs=2, space="PSUM") as pmm, \
             tc.tile_pool(name="pss", bufs=1, space="PSUM") as pss:

            h_sb0 = keep.tile([128, NT, HID], bf16)     # h0 node-major local
            ag_sb = keep.tile([128, NT, HID], f8)       # h0 fp8 (AG staging)
            h0a = keep.tile([128, ST, HID], f8)         # gathered global h0
            hT = keep.tile([128, 6, PROWS], bf16)       # feature-major concat
            wout_sb = keep.tile([128, BLOBB], bf16)
            pa_t = keep.tile([128, BLOBA], bf16)

            nc.sync.dma_start(wout_sb[:], blob_b[:])
            ident = wout_sb[:, OI:OI + 128]

            ag_in0 = dram.tile([128, HT0 * HID], f8)
            ag_in1 = dram.tile([128, (NT - HT0) * HID], f8)
            ag_out0 = dram.tile([NCORES * 128, HT0 * HID], f8,
                                addr_space="Shared")
            ag_out1 = dram.tile([NCORES * 128, (NT - HT0) * HID], f8,
                                addr_space="Shared")

            # ---- phase A: h0 = x @ W1 (bf16), row-tile pipelined ----
            # Bulk streams (x, A) go through the Activation HWDGE queue;
            # latency-critical small DMAs stay on the SP (sync) queue.
            with nc.named_scope("h0_gemm"):
                nc.sync.dma_start(pa_t[:, OW1:OW1 + KT * HID],
                                  blob_a[:, OW1:OW1 + KT * HID])
                for t in range(NT):
                    o = OX + t * KT * 128
                    if t < 2:
                        nc.scalar.dma_start(pa_t[:, o:o + KT * 128],
                                            blob_a[:, o:o + KT * 128])
                for t in range(NT):
                    if t + 2 < NT:
                        o2 = OX + (t + 2) * KT * 128
                        nc.scalar.dma_start(pa_t[:, o2:o2 + KT * 128],
                                            blob_a[:, o2:o2 + KT * 128])
                    ps = pmm.tile([128, HID], f32, tag="mm")
                    o = OX + t * KT * 128
                    for k in range(KT):
                        nc.tensor.matmul(
                            ps[:],
                            pa_t[:, o + k * 128:o + (k + 1) * 128],
                            pa_t[:, OW1 + k * HID:OW1 + (k + 1) * HID],
                            start=(k == 0), stop=(k == KT - 1),
                        )
                    nc.vector.tensor_copy(h_sb0[:, t, :], ps[:])
                    nc.vector.tensor_copy(ag_sb[:, t, :], ps[:])
                    for half in range(2):
                        pt = pmm.tile([128, HID], f32, tag="mm")
                        ptb = pt[:].bitcast(bf16)[:, 0:128]
                        nc.tensor.transpose(
                            ptb, h_sb0[:, t, 128 * half:128 * (half + 1)],
                            ident)
                        nc.vector.tensor_copy(
                            hT[:, half, 128 * t:128 * (t + 1)], ptb)
                    if t == HT0 - 1:
                        nc.sync.dma_start(
                            ag_in0[:].rearrange("p (a m) -> p a m", a=HT0),
                            ag_sb[:, 0:HT0, :])
                    if t == NT - 1:
                        nc.scalar.dma_start(
                            ag_in1[:].rearrange("p (a m) -> p a m", a=NT - HT0),
                            ag_sb[:, HT0:NT, :])

            # ---- phase B: AllGather h0 (fp8), two chunks, then unpack ----
            with nc.named_scope("allgather"):
                nc.gpsimd.collective_compute(
                    "AllGather", mybir.AluOpType.bypass,
                    replica_groups=[list(range(NCORES))],
                    ins=[ag_in0.opt()], outs=[ag_out0.opt()],
                )
                nc.gpsimd.collective_compute(
                    "AllGather", mybir.AluOpType.bypass,
                    replica_groups=[list(range(NCORES))],
                    ins=[ag_in1.opt()], outs=[ag_out1.opt()],
                )
                for r in range(NCORES):
                    nc.sync.dma_start(
                        h0a[:, r * NT:r * NT + HT0, :],
                        ag_out0[r * 128:(r + 1) * 128, :]
                        .rearrange("p (a m) -> p a m", a=HT0))
                for r in range(NCORES):
                    nc.sync.dma_start(
                        h0a[:, r * NT + HT0:(r + 1) * NT, :],
                        ag_out1[r * 128:(r + 1) * 128, :]
                        .rearrange("p (a m) -> p a m", a=NT - HT0))

            # ---- phase C: SpMM, fp8 DoubleRow, A moving / h0 stationary ----
            # out[feat, dest] += sum_src h0a[src, feat] * A[src, dest]
            with nc.named_scope("spmm"):
                DCH = (512, 512, 256)
                for a in range(2):
                    ps6 = [[pss.tile([128, 512], f32, tag=f"s{fh}{d}",
                                     name=f"ps_s{fh}{d}")
                            for d in range(3)] for fh in range(2)]
                    for gi, g in enumerate(G_ORDER):
                        at = keep.tile([128, 2, PROWS], f8, tag="a", bufs=12)
                        off = (a * NG + g) * 2 * PROWS
                        nc.scalar.dma_start(
                            at[:], A_d[:, off:off + 2 * PROWS]
                            .rearrange("p (a m) -> p a m", a=2))
                        for fh in range(2):
                            lhs = h0a[:, 2 * g:2 * g + 2,
                                      128 * fh:128 * (fh + 1)]
                            dpos = 0
                            for d, w in enumerate(DCH):
                                nc.tensor.matmul(
                                    ps6[fh][d][:, 0:w], lhs,
                                    at[:, :, dpos:dpos + w],
                                    start=(gi == 0), stop=(gi == NG - 1),
                                    perf_mode=DR,
                                )
                                dpos += w
                    for fh in range(2):
                        dpos = 0
                        for d, w in enumerate(DCH):
                            nc.vector.tensor_copy(
                                hT[:, 2 + 2 * a + fh, dpos:dpos + w],
                                ps6[fh][d][:, 0:w])
                            dpos += w

            # ---- phase D: out = h @ Wout + b (bf16) ----
            with nc.named_scope("out_gemm"):
                for t in range(NT):
                    ps = pmm.tile([128, OUT_C], f32, tag="mm")
                    nc.tensor.matmul(ps[:], wout_sb[0:1, OO:OO + 128],
                                     wout_sb[0:1, OB:OB + OUT_C],
                                     start=True, stop=False)
                    for k in range(6):
                        nc.tensor.matmul(
                            ps[:],
                            hT[:, k, 128 * t:128 * (t + 1)],
                            wout_sb[:, OWO + k * OUT_C:OWO + (k + 1) * OUT_C],
                            start=False, stop=(k == 5),
                        )
                    o_sb = keep.tile([128, OUT_C], f32, tag="osb", bufs=2)
                    nc.vector.tensor_copy(o_sb[:], ps[:])
                    rows = min(128, ROWS - 128 * t)
                    nc.sync.dma_start(out[128 * t:128 * t + rows, :],
                                      o_sb[:rows, :])
    nc.compile()
    return nc


def kernel(x, adj1_rows, adj1_cols, adj1_vals, adj2_rows, adj2_cols, adj2_vals,
           W1, W_out, b_out):
    global LAST_EXEC_NS, LAST_RESULTS
    _install_trace_shim()
    x = np.asarray(x, np.float32)
    W1 = np.ascontiguousarray(np.asarray(W1, np.float32))
    W_out = np.ascontiguousarray(np.asarray(W_out, np.float32)).copy()
    b_out = np.asarray(b_out, np.float32).ravel()
    # undo the fp8-range pre-scaling of A1/A2 in the matching W_out rows
    W_out[HID:2 * HID] *= 1.0 / 16.0
    W_out[2 * HID:3 * HID] *= 1.0 / 32.0

    w1_cols = W1.reshape(KT, 128, HID).transpose(1, 0, 2).reshape(128, KT * HID)
    blob_b = np.zeros((128, BLOBB), np.float32)
    blob_b[:, OWO:OWO + 6 * OUT_C] = \
        W_out.reshape(6, 128, OUT_C).transpose(1, 0, 2).reshape(128, 6 * OUT_C)
    blob_b[0, OB:OB + OUT_C] = b_out
    blob_b[0, OO:OO + 128] = 1.0
    blob_b[:, OI:OI + 128] = np.eye(128, dtype=np.float32)
    blob_b = blob_b.astype(bfnp)

    a1r = np.asarray(adj1_rows, np.int64)
    a1c = np.asarray(adj1_cols, np.int64)
    a1v = np.asarray(adj1_vals, np.float32)
    a2r = np.asarray(adj2_rows, np.int64)
    a2c = np.asarray(adj2_cols, np.int64)
    a2v = np.asarray(adj2_vals, np.float32)

    in_maps = []
    for c in range(NCORES):
        xtp = np.zeros((IN_C, PROWS), np.float32)
        xtp[:, :ROWS] = x[c * ROWS:(c + 1) * ROWS].T
        blob_a = np.concatenate([
            w1_cols,
            xtp.reshape(KT, 128, NT, 128).transpose(1, 2, 0, 3)
            .reshape(128, NT * KT * 128),
        ], axis=1).astype(bfnp)
        A_pack = np.concatenate([
            _pack_adj(a1r, a1c, a1v, c, 16.0),
            _pack_adj(a2r, a2c, a2v, c, 32.0),
        ], axis=1)
        in_maps.append({"blob_a": blob_a, "blob_b": blob_b, "A_d": A_pack})

    nc = _build()
    try:
        res = bass_utils.run_bass_kernel_spmd(
            nc, in_maps, core_ids=list(range(NCORES)), trace=True,
            trace_cores=[0])
    except Exception:
        res = bass_utils.run_bass_kernel_spmd(
            nc, in_maps, core_ids=list(range(NCORES)), trace=False)
    LAST_EXEC_NS = res.exec_time_ns
    LAST_RESULTS = res
    return np.concatenate([res.results[c]["out"] for c in range(NCORES)], axis=0)


# revision 22
# speedup vs baseline: 1.3652x; 1.3652x over previous
"""H2GCN forward on 8 Trainium2 NeuronCores.

out = concat([h0, A1@h0, A2@h0], 1) @ W_out + b_out,  h0 = x @ W1

Data-parallel over destination nodes (1250 rows/core, padded to 1280).
Per core: h0 = x@W1 in bf16 (row-tile pipelined with the x DMA), h0
quantized to fp8-e4m3 and AllGathered in two chunks, SpMM as dense
fp8 DoubleRow matmuls with the adjacency blocks as the MOVING operand
and h0 tiles stationary -- so 256 sources contract per instruction and
the output lands feature-major (no transpose phase for h1/h2).  A1 is
pre-scaled by 16 and A2 by 32 (undone in W_out rows) to keep edge
weights in fp8's normal range.  Final GEMM in bf16 with bias as a K=1
matmul.
"""
import sys
import types

for _p in ("/opt/trn_rl_repo", "/root/.axon_site", "/root/.axon_site/_ro/trn_rl_repo",
           "/root/.axon_site/_ro/pypackages"):
    if _p not in sys.path:
        sys.path.append(_p)

import numpy as np
import ml_dtypes
import concourse.bass as bass
import concourse.bacc as bacc
import concourse.mybir as mybir
import concourse.tile as tile
from concourse import bass_utils

N, IN_C, HID, OUT_C = 10000, 2048, 256, 256
NCORES = 8
ROWS = N // NCORES          # 1250
PROWS = 1280                # padded (10 x 128)
NT = PROWS // 128           # 10 row tiles per core
KT = IN_C // 128            # 16 k tiles
ST = NCORES * NT            # 80 source tiles in the padded gather space
NG = ST // 2                # 40 source super-tiles (DoubleRow pairs)
CH = [(0, 4), (4, 8), (8, 10)]   # AllGather chunk tile ranges

f32 = mybir.dt.float32
bf16 = mybir.dt.bfloat16
f8 = mybir.dt.float8e4
bfnp = ml_dtypes.bfloat16
f8np = ml_dtypes.float8_e4m3fn

# blob_a (bf16): W1 k-tiles then x row-tile-major k-tiles
OW1, OX = 0, KT * HID
BLOBA = KT * HID + NT * KT * 128
# blob_b (bf16): Wout k-tiles, bias (row 0), ones (row 0), identity
OWO, OB, OO, OI = 0, 6 * OUT_C, 6 * OUT_C + OUT_C, 6 * OUT_C + OUT_C + 128
BLOBB = OI + 128

# spmm processing order: super-tiles grouped by AllGather chunk
G_ORDER = [r * (NT // 2) + j for lo, hi in CH
           for r in range(NCORES) for j in range(lo // 2, hi // 2)]

LAST_EXEC_NS = None
LAST_RESULTS = None


def _install_trace_shim():
    try:
        import antenv.axon_hooks  # noqa: F401
        return
    except ImportError:
        pass
    try:
        import antenv
        from trn_agent_boot.trn_boot import _ntff_profile_via_ctypes
        hook = _ntff_profile_via_ctypes("/opt/axon/libaxon_pjrt.so")
        mod = types.ModuleType("antenv.axon_hooks")
        mod.get_axon_ntff_profile_hook = lambda: hook
        mod.set_axon_ntff_profile_hook = lambda h: None
        sys.modules["antenv.axon_hooks"] = mod
        antenv.axon_hooks = mod
    except Exception:
        pass


def _pack_adj(rows, cols, vals, core, scale):
    """fp8 dense A^T for this core's dest shard, laid out
    [128 src-part, NG super, 2 ktile, PROWS dest] (flattened free dim)."""
    lo, hi = core * ROWS, (core + 1) * ROWS
    m = (rows >= lo) & (rows < hi)
    r, c, v = rows[m] - lo, cols[m], vals[m] * scale
    A = np.zeros((NCORES * PROWS, PROWS), np.float32)
    src = (c // ROWS) * PROWS + (c % ROWS)
    np.add.at(A, (src, r), v)
    return np.ascontiguousarray(
        A.reshape(NG, 2, 128, PROWS).transpose(2, 0, 1, 3)
        .reshape(128, NG * 2 * PROWS)).astype(f8np)


def _build():
    nc = bacc.Bacc("TRN2", target_bir_lowering=False, debug=False,
                   num_devices=8)
    blob_a = nc.dram_tensor("blob_a", [128, BLOBA], bf16, kind="ExternalInput")
    blob_b = nc.dram_tensor("blob_b", [128, BLOBB], bf16, kind="ExternalInput")
    A_d = nc.dram_tensor("A_d", [128, 2 * NG * 2 * PROWS], f8,
                         kind="ExternalInput")
    out = nc.dram_tensor("out", [ROWS, OUT_C], f32, kind="ExternalOutput")

    DR = mybir.MatmulPerfMode.DoubleRow

    with tile.TileContext(nc) as tc:
        with tc.tile_pool(name="keep", bufs=1) as keep, \
             tc.tile_pool(name="dram", bufs=1, space="DRAM") as dram, \
             tc.tile_pool(name="pmm", bufs=2, space="PSUM") as pmm, \
             tc.tile_pool(name="pss", bufs=1, space="PSUM") as pss:

            h_sb0 = keep.tile([128, NT, HID], bf16)     # h0 node-major local
            ag_sb = keep.tile([128, NT, HID], f8)       # h0 fp8 (AG staging)
            h0a = keep.tile([128, ST, HID], f8)         # gathered global h0
            hT = keep.tile([128, 6, PROWS], bf16)       # feature-major concat
            wout_sb = keep.tile([128, BLOBB], bf16)
            pa_t = keep.tile([128, BLOBA], bf16)

            nc.sync.dma_start(wout_sb[:], blob_b[:])
            ident = wout_sb[:, OI:OI + 128]

            ag_ins, ag_outs = [], []
            for ci, (lo, hi) in enumerate(CH):
                w = hi - lo
                ag_ins.append(dram.tile([128, w * HID], f8,
                                        name=f"ag_in{ci}"))
                ag_outs.append(dram.tile([NCORES * 128, w * HID], f8,
                                         addr_space="Shared",
                                         name=f"ag_out{ci}"))

            # ---- phase A: h0 = x @ W1 (bf16), row-tile pipelined ----
            # Bulk streams (x, A) go through the Activation HWDGE queue;
            # latency-critical small DMAs stay on the SP (sync) queue.
            with nc.named_scope("h0_gemm"):
                nc.sync.dma_start(pa_t[:, OW1:OW1 + KT * HID],
                                  blob_a[:, OW1:OW1 + KT * HID])
                for t in range(NT):
                    o = OX + t * KT * 128
                    if t < 2:
                        nc.scalar.dma_start(pa_t[:, o:o + KT * 128],
                                            blob_a[:, o:o + KT * 128])
                for t in range(NT):
                    if t + 2 < NT:
                        o2 = OX + (t + 2) * KT * 128
                        nc.scalar.dma_start(pa_t[:, o2:o2 + KT * 128],
                                            blob_a[:, o2:o2 + KT * 128])
                    ps = pmm.tile([128, HID], f32, tag="mm")
                    o = OX + t * KT * 128
                    for k in range(KT):
                        nc.tensor.matmul(
                            ps[:],
                            pa_t[:, o + k * 128:o + (k + 1) * 128],
                            pa_t[:, OW1 + k * HID:OW1 + (k + 1) * HID],
                            start=(k == 0), stop=(k == KT - 1),
                        )
                    nc.vector.tensor_copy(h_sb0[:, t, :], ps[:])
                    nc.vector.tensor_copy(ag_sb[:, t, :], ps[:])
                    for half in range(2):
                        pt = pmm.tile([128, HID], f32, tag="mm")
                        ptb = pt[:].bitcast(bf16)[:, 0:128]
                        nc.tensor.transpose(
                            ptb, h_sb0[:, t, 128 * half:128 * (half + 1)],
                            ident)
                        nc.vector.tensor_copy(
                            hT[:, half, 128 * t:128 * (t + 1)], ptb)
                    for ci, (lo, hi) in enumerate(CH):
                        if t == hi - 1:
                            nc.sync.dma_start(
                                ag_ins[ci][:].rearrange(
                                    "p (a m) -> p a m", a=hi - lo),
                                ag_sb[:, lo:hi, :])

            # ---- phase B: AllGather h0 (fp8), three chunks, then unpack ----
            with nc.named_scope("allgather"):
                for ci, (lo, hi) in enumerate(CH):
                    nc.gpsimd.collective_compute(
                        "AllGather", mybir.AluOpType.bypass,
                        replica_groups=[list(range(NCORES))],
                        ins=[ag_ins[ci].opt()], outs=[ag_outs[ci].opt()],
                    )
                for ci, (lo, hi) in enumerate(CH):
                    for r in range(NCORES):
                        nc.sync.dma_start(
                            h0a[:, r * NT + lo:r * NT + hi, :],
                            ag_outs[ci][r * 128:(r + 1) * 128, :]
                            .rearrange("p (a m) -> p a m", a=hi - lo))

            # ---- phase C: SpMM, fp8 DoubleRow, A moving / h0 stationary ----
            # out[feat, dest] += sum_src h0a[src, feat] * A[src, dest]
            with nc.named_scope("spmm"):
                DCH = (512, 512, 256)
                for a in range(2):
                    ps6 = [[pss.tile([128, 512], f32, tag=f"s{fh}{d}",
                                     name=f"ps_s{fh}{d}")
                            for d in range(3)] for fh in range(2)]
                    for gi, g in enumerate(G_ORDER):
                        at = keep.tile([128, 2, PROWS], f8, tag="a", bufs=12)
                        off = (a * NG + g) * 2 * PROWS
                        nc.scalar.dma_start(
                            at[:], A_d[:, off:off + 2 * PROWS]
                            .rearrange("p (a m) -> p a m", a=2))
                        for fh in range(2):
                            lhs = h0a[:, 2 * g:2 * g + 2,
                                      128 * fh:128 * (fh + 1)]
                            dpos = 0
                            for d, w in enumerate(DCH):
                                nc.tensor.matmul(
                                    ps6[fh][d][:, 0:w], lhs,
                                    at[:, :, dpos:dpos + w],
                                    start=(gi == 0), stop=(gi == NG - 1),
                                    perf_mode=DR,
                                )
                                dpos += w
                    for fh in range(2):
                        dpos = 0
                        for d, w in enumerate(DCH):
                            nc.vector.tensor_copy(
                                hT[:, 2 + 2 * a + fh, dpos:dpos + w],
                                ps6[fh][d][:, 0:w])
                            dpos += w

            # ---- phase D: out = h @ Wout + b (bf16) ----
            with nc.named_scope("out_gemm"):
                for t in range(NT):
                    ps = pmm.tile([128, OUT_C], f32, tag="mm")
                    nc.tensor.matmul(ps[:], wout_sb[0:1, OO:OO + 128],
                                     wout_sb[0:1, OB:OB + OUT_C],
                                     start=True, stop=False)
                    for k in range(6):
                        nc.tensor.matmul(
                            ps[:],
                            hT[:, k, 128 * t:128 * (t + 1)],
                            wout_sb[:, OWO + k * OUT_C:OWO + (k + 1) * OUT_C],
                            start=False, stop=(k == 5),
                        )
                    o_sb = keep.tile([128, OUT_C], f32, tag="osb", bufs=2)
                    nc.vector.tensor_copy(o_sb[:], ps[:])
                    rows = min(128, ROWS - 128 * t)
                    nc.sync.dma_start(out[128 * t:128 * t + rows, :],
                                      o_sb[:rows, :])
    nc.compile()
    return nc


def kernel(x, adj1_rows, adj1_cols, adj1_vals, adj2_rows, adj2_cols, adj2_vals,
           W1, W_out, b_out):
    global LAST_EXEC_NS, LAST_RESULTS
    _install_trace_shim()
    x = np.asarray(x, np.float32)
    W1 = np.ascontiguousarray(np.asarray(W1, np.float32))
    W_out = np.ascontiguousarray(np.asarray(W_out, np.float32)).copy()
    b_out = np.asarray(b_out, np.float32).ravel()
    # undo the fp8-range pre-scaling of A1/A2 in the matching W_out rows
    W_out[HID:2 * HID] *= 1.0 / 16.0
    W_out[2 * HID:3 * HID] *= 1.0 / 32.0

    w1_cols = W1.reshape(KT, 128, HID).transpose(1, 0, 2).reshape(128, KT * HID)
    blob_b = np.zeros((128, BLOBB), np.float32)
    blob_b[:, OWO:OWO + 6 * OUT_C] = \
        W_out.reshape(6, 128, OUT_C).transpose(1, 0, 2).reshape(128, 6 * OUT_C)
    blob_b[0, OB:OB + OUT_C] = b_out
    blob_b[0, OO:OO + 128] = 1.0
    blob_b[:, OI:OI + 128] = np.eye(128, dtype=np.float32)
    blob_b = blob_b.astype(bfnp)

    a1r = np.asarray(adj1_rows, np.int64)
    a1c = np.asarray(adj1_cols, np.int64)
    a1v = np.asarray(adj1_vals, np.float32)
    a2r = np.asarray(adj2_rows, np.int64)
    a2c = np.asarray(adj2_cols, np.int64)
    a2v = np.asarray(adj2_vals, np.float32)

    in_maps = []
    for c in range(NCORES):
        xtp = np.zeros((IN_C, PROWS), np.float32)
        xtp[:, :ROWS] = x[c * ROWS:(c + 1) * ROWS].T
        blob_a = np.concatenate([
            w1_cols,
            xtp.reshape(KT, 128, NT, 128).transpose(1, 2, 0, 3)
            .reshape(128, NT * KT * 128),
        ], axis=1).astype(bfnp)
        A_pack = np.concatenate([
            _pack_adj(a1r, a1c, a1v, c, 16.0),
            _pack_adj(a2r, a2c, a2v, c, 32.0),
        ], axis=1)
        in_maps.append({"blob_a": blob_a, "blob_b": blob_b, "A_d": A_pack})

    nc = _build()
    try:
        res = bass_utils.run_bass_kernel_spmd(
            nc, in_maps, core_ids=list(range(NCORES)), trace=True,
            trace_cores=[0])
    except Exception:
        res = bass_utils.run_bass_kernel_spmd(
            nc, in_maps, core_ids=list(range(NCORES)), trace=False)
    LAST_EXEC_NS = res.exec_time_ns
    LAST_RESULTS = res
    return np.concatenate([res.results[c]["out"] for c in range(NCORES)], axis=0)


# revision 23
# speedup vs baseline: 1.4205x; 1.0404x over previous
"""H2GCN forward on 8 Trainium2 NeuronCores.

out = concat([h0, A1@h0, A2@h0], 1) @ W_out + b_out,  h0 = x @ W1

Data-parallel over destination nodes (1250 rows/core, padded to 1280).
Per core: h0 = x@W1 in bf16 (row-tile pipelined with the x DMA), h0
quantized to fp8-e4m3 and AllGathered in two chunks, SpMM as dense
fp8 DoubleRow matmuls with the adjacency blocks as the MOVING operand
and h0 tiles stationary -- so 256 sources contract per instruction and
the output lands feature-major (no transpose phase for h1/h2).  A1 is
pre-scaled by 16 and A2 by 32 (undone in W_out rows) to keep edge
weights in fp8's normal range.  Final GEMM in bf16 with bias as a K=1
matmul.
"""
import sys
import types

for _p in ("/opt/trn_rl_repo", "/root/.axon_site", "/root/.axon_site/_ro/trn_rl_repo",
           "/root/.axon_site/_ro/pypackages"):
    if _p not in sys.path:
        sys.path.append(_p)

import numpy as np
import ml_dtypes
import concourse.bass as bass
import concourse.bacc as bacc
import concourse.mybir as mybir
import concourse.tile as tile
from concourse import bass_utils

N, IN_C, HID, OUT_C = 10000, 2048, 256, 256
NCORES = 8
ROWS = N // NCORES          # 1250
PROWS = 1280                # padded (10 x 128)
NT = PROWS // 128           # 10 row tiles per core
KT = IN_C // 128            # 16 k tiles
ST = NCORES * NT            # 80 source tiles in the padded gather space
NG = ST // 2                # 40 source super-tiles (DoubleRow pairs)
CH = [(0, 8), (8, 10)]   # AllGather chunk tile ranges

f32 = mybir.dt.float32
bf16 = mybir.dt.bfloat16
f8 = mybir.dt.float8e4
bfnp = ml_dtypes.bfloat16
f8np = ml_dtypes.float8_e4m3fn

# blob_a (bf16): W1 k-tiles then x row-tile-major k-tiles
OW1, OX = 0, KT * HID
BLOBA = KT * HID + NT * KT * 128
# blob_b (bf16): Wout k-tiles, bias (row 0), ones (row 0), identity
OWO, OB, OO, OI = 0, 6 * OUT_C, 6 * OUT_C + OUT_C, 6 * OUT_C + OUT_C + 128
BLOBB = OI + 128

# spmm processing order: super-tiles grouped by AllGather chunk
G_ORDER = [r * (NT // 2) + j for lo, hi in CH
           for r in range(NCORES) for j in range(lo // 2, hi // 2)]

LAST_EXEC_NS = None
LAST_RESULTS = None


def _install_trace_shim():
    try:
        import antenv.axon_hooks  # noqa: F401
        return
    except ImportError:
        pass
    try:
        import antenv
        from trn_agent_boot.trn_boot import _ntff_profile_via_ctypes
        hook = _ntff_profile_via_ctypes("/opt/axon/libaxon_pjrt.so")
        mod = types.ModuleType("antenv.axon_hooks")
        mod.get_axon_ntff_profile_hook = lambda: hook
        mod.set_axon_ntff_profile_hook = lambda h: None
        sys.modules["antenv.axon_hooks"] = mod
        antenv.axon_hooks = mod
    except Exception:
        pass


def _pack_adj(rows, cols, vals, core, scale):
    """fp8 dense A^T for this core's dest shard, laid out
    [128 src-part, NG super, 2 ktile, PROWS dest] (flattened free dim)."""
    lo, hi = core * ROWS, (core + 1) * ROWS
    m = (rows >= lo) & (rows < hi)
    r, c, v = rows[m] - lo, cols[m], vals[m] * scale
    A = np.zeros((NCORES * PROWS, PROWS), np.float32)
    src = (c // ROWS) * PROWS + (c % ROWS)
    np.add.at(A, (src, r), v)
    return np.ascontiguousarray(
        A.reshape(NG, 2, 128, PROWS).transpose(2, 0, 1, 3)
        .reshape(128, NG * 2 * PROWS)).astype(f8np)


def _build():
    nc = bacc.Bacc("TRN2", target_bir_lowering=False, debug=False,
                   num_devices=8)
    blob_a = nc.dram_tensor("blob_a", [128, BLOBA], bf16, kind="ExternalInput")
    blob_b = nc.dram_tensor("blob_b", [128, BLOBB], bf16, kind="ExternalInput")
    A_d = nc.dram_tensor("A_d", [128, 2 * NG * 2 * PROWS], f8,
                         kind="ExternalInput")
    out = nc.dram_tensor("out", [ROWS, OUT_C], f32, kind="ExternalOutput")

    DR = mybir.MatmulPerfMode.DoubleRow

    with tile.TileContext(nc) as tc:
        with tc.tile_pool(name="keep", bufs=1) as keep, \
             tc.tile_pool(name="dram", bufs=1, space="DRAM") as dram, \
             tc.tile_pool(name="pmm", bufs=2, space="PSUM") as pmm, \
             tc.tile_pool(name="pss", bufs=1, space="PSUM") as pss:

            h_sb0 = keep.tile([128, NT, HID], bf16)     # h0 node-major local
            ag_sb = keep.tile([128, NT, HID], f8)       # h0 fp8 (AG staging)
            h0a = keep.tile([128, ST, HID], f8)         # gathered global h0
            hT = keep.tile([128, 6, PROWS], bf16)       # feature-major concat
            wout_sb = keep.tile([128, BLOBB], bf16)
            pa_t = keep.tile([128, BLOBA], bf16)

            nc.sync.dma_start(wout_sb[:], blob_b[:])
            ident = wout_sb[:, OI:OI + 128]

            ag_ins, ag_outs = [], []
            for ci, (lo, hi) in enumerate(CH):
                w = hi - lo
                ag_ins.append(dram.tile([128, w * HID], f8,
                                        name=f"ag_in{ci}"))
                ag_outs.append(dram.tile([NCORES * 128, w * HID], f8,
                                         addr_space="Shared",
                                         name=f"ag_out{ci}"))

            # ---- phase A: h0 = x @ W1 (bf16), row-tile pipelined ----
            # Bulk streams (x, A) go through the Activation HWDGE queue;
            # latency-critical small DMAs stay on the SP (sync) queue.
            with nc.named_scope("h0_gemm"):
                nc.sync.dma_start(pa_t[:, OW1:OW1 + KT * HID],
                                  blob_a[:, OW1:OW1 + KT * HID])
                for t in range(NT):
                    o = OX + t * KT * 128
                    if t < 2:
                        nc.scalar.dma_start(pa_t[:, o:o + KT * 128],
                                            blob_a[:, o:o + KT * 128])
                for t in range(NT):
                    if t + 2 < NT:
                        o2 = OX + (t + 2) * KT * 128
                        nc.scalar.dma_start(pa_t[:, o2:o2 + KT * 128],
                                            blob_a[:, o2:o2 + KT * 128])
                    ps = pmm.tile([128, HID], f32, tag="mm")
                    o = OX + t * KT * 128
                    for k in range(KT):
                        nc.tensor.matmul(
                            ps[:],
                            pa_t[:, o + k * 128:o + (k + 1) * 128],
                            pa_t[:, OW1 + k * HID:OW1 + (k + 1) * HID],
                            start=(k == 0), stop=(k == KT - 1),
                        )
                    nc.vector.tensor_copy(h_sb0[:, t, :], ps[:])
                    nc.vector.tensor_copy(ag_sb[:, t, :], ps[:])
                    for half in range(2):
                        pt = pmm.tile([128, HID], f32, tag="mm")
                        ptb = pt[:].bitcast(bf16)[:, 0:128]
                        nc.tensor.transpose(
                            ptb, h_sb0[:, t, 128 * half:128 * (half + 1)],
                            ident)
                        nc.vector.tensor_copy(
                            hT[:, half, 128 * t:128 * (t + 1)], ptb)
                    for ci, (lo, hi) in enumerate(CH):
                        if t == hi - 1:
                            nc.sync.dma_start(
                                ag_ins[ci][:].rearrange(
                                    "p (a m) -> p a m", a=hi - lo),
                                ag_sb[:, lo:hi, :])

            # ---- phase B: AllGather h0 (fp8), three chunks, then unpack ----
            with nc.named_scope("allgather"):
                for ci, (lo, hi) in enumerate(CH):
                    nc.gpsimd.collective_compute(
                        "AllGather", mybir.AluOpType.bypass,
                        replica_groups=[list(range(NCORES))],
                        ins=[ag_ins[ci].opt()], outs=[ag_outs[ci].opt()],
                    )
                for ci, (lo, hi) in enumerate(CH):
                    for r in range(NCORES):
                        nc.sync.dma_start(
                            h0a[:, r * NT + lo:r * NT + hi, :],
                            ag_outs[ci][r * 128:(r + 1) * 128, :]
                            .rearrange("p (a m) -> p a m", a=hi - lo))

            # ---- phase C: SpMM, fp8 DoubleRow, A moving / h0 stationary ----
            # out[feat, dest] += sum_src h0a[src, feat] * A[src, dest]
            with nc.named_scope("spmm"):
                DCH = (512, 512, 256)
                for a in range(2):
                    ps6 = [[pss.tile([128, 512], f32, tag=f"s{fh}{d}",
                                     name=f"ps_s{fh}{d}")
                            for d in range(3)] for fh in range(2)]
                    for gi, g in enumerate(G_ORDER):
                        at = keep.tile([128, 2, PROWS], f8, tag="a", bufs=12)
                        off = (a * NG + g) * 2 * PROWS
                        nc.scalar.dma_start(
                            at[:], A_d[:, off:off + 2 * PROWS]
                            .rearrange("p (a m) -> p a m", a=2))
                        for fh in range(2):
                            lhs = h0a[:, 2 * g:2 * g + 2,
                                      128 * fh:128 * (fh + 1)]
                            dpos = 0
                            for d, w in enumerate(DCH):
                                nc.tensor.matmul(
                                    ps6[fh][d][:, 0:w], lhs,
                                    at[:, :, dpos:dpos + w],
                                    start=(gi == 0), stop=(gi == NG - 1),
                                    perf_mode=DR,
                                )
                                dpos += w
                    for fh in range(2):
                        dpos = 0
                        for d, w in enumerate(DCH):
                            nc.vector.tensor_copy(
                                hT[:, 2 + 2 * a + fh, dpos:dpos + w],
                                ps6[fh][d][:, 0:w])
                            dpos += w

            # ---- phase D: out = h @ Wout + b (bf16) ----
            with nc.named_scope("out_gemm"):
                for t in range(NT):
                    ps = pmm.tile([128, OUT_C], f32, tag="mm")
                    nc.tensor.matmul(ps[:], wout_sb[0:1, OO:OO + 128],
                                     wout_sb[0:1, OB:OB + OUT_C],
                                     start=True, stop=False)
                    for k in range(6):
                        nc.tensor.matmul(
                            ps[:],
                            hT[:, k, 128 * t:128 * (t + 1)],
                            wout_sb[:, OWO + k * OUT_C:OWO + (k + 1) * OUT_C],
                            start=False, stop=(k == 5),
                        )
                    o_sb = keep.tile([128, OUT_C], f32, tag="osb", bufs=2)
                    nc.vector.tensor_copy(o_sb[:], ps[:])
                    rows = min(128, ROWS - 128 * t)
                    nc.sync.dma_start(out[128 * t:128 * t + rows, :],
                                      o_sb[:rows, :])
    nc.compile()
    return nc


def kernel(x, adj1_rows, adj1_cols, adj1_vals, adj2_rows, adj2_cols, adj2_vals,
           W1, W_out, b_out):
    global LAST_EXEC_NS, LAST_RESULTS
    _install_trace_shim()
    x = np.asarray(x, np.float32)
    W1 = np.ascontiguousarray(np.asarray(W1, np.float32))
    W_out = np.ascontiguousarray(np.asarray(W_out, np.float32)).copy()
    b_out = np.asarray(b_out, np.float32).ravel()
    # undo the fp8-range pre-scaling of A1/A2 in the matching W_out rows
    W_out[HID:2 * HID] *= 1.0 / 16.0
    W_out[2 * HID:3 * HID] *= 1.0 / 32.0

    w1_cols = W1.reshape(KT, 128, HID).transpose(1, 0, 2).reshape(128, KT * HID)
    blob_b = np.zeros((128, BLOBB), np.float32)
    blob_b[:, OWO:OWO + 6 * OUT_C] = \
        W_out.reshape(6, 128, OUT_C).transpose(1, 0, 2).reshape(128, 6 * OUT_C)
    blob_b[0, OB:OB + OUT_C] = b_out
    blob_b[0, OO:OO + 128] = 1.0
    blob_b[:, OI:OI + 128] = np.eye(128, dtype=np.float32)
    blob_b = blob_b.astype(bfnp)

    a1r = np.asarray(adj1_rows, np.int64)
    a1c = np.asarray(adj1_cols, np.int64)
    a1v = np.asarray(adj1_vals, np.float32)
    a2r = np.asarray(adj2_rows, np.int64)
    a2c = np.asarray(adj2_cols, np.int64)
    a2v = np.asarray(adj2_vals, np.float32)

    in_maps = []
    for c in range(NCORES):
        xtp = np.zeros((IN_C, PROWS), np.float32)
        xtp[:, :ROWS] = x[c * ROWS:(c + 1) * ROWS].T
        blob_a = np.concatenate([
            w1_cols,
            xtp.reshape(KT, 128, NT, 128).transpose(1, 2, 0, 3)
            .reshape(128, NT * KT * 128),
        ], axis=1).astype(bfnp)
        A_pack = np.concatenate([
            _pack_adj(a1r, a1c, a1v, c, 16.0),
            _pack_adj(a2r, a2c, a2v, c, 32.0),
        ], axis=1)
        in_maps.append({"blob_a": blob_a, "blob_b": blob_b, "A_d": A_pack})

    nc = _build()
    try:
        res = bass_utils.run_bass_kernel_spmd(
            nc, in_maps, core_ids=list(range(NCORES)), trace=True,
            trace_cores=[0])
    except Exception:
        res = bass_utils.run_bass_kernel_spmd(
            nc, in_maps, core_ids=list(range(NCORES)), trace=False)
    LAST_EXEC_NS = res.exec_time_ns
    LAST_RESULTS = res
    return np.concatenate([res.results[c]["out"] for c in range(NCORES)], axis=0)


# revision 25
# speedup vs baseline: 1.5056x; 1.0600x over previous
"""H2GCN forward on 8 Trainium2 NeuronCores.

out = concat([h0, A1@h0, A2@h0], 1) @ W_out + b_out,  h0 = x @ W1

Data-parallel over destination nodes (1250 rows/core, padded to 1280).
Per core: h0 = x@W1 in bf16 (row-tile pipelined with the x DMA), h0
quantized to fp8-e4m3 and AllGathered in two chunks, SpMM as dense
fp8 DoubleRow matmuls with the adjacency blocks as the MOVING operand
and h0 tiles stationary -- so 256 sources contract per instruction and
the output lands feature-major (no transpose phase for h1/h2).  A1 is
pre-scaled by 16 and A2 by 32 (undone in W_out rows) to keep edge
weights in fp8's normal range.  Final GEMM in bf16 with bias as a K=1
matmul.
"""
import sys
import types

for _p in ("/opt/trn_rl_repo", "/root/.axon_site", "/root/.axon_site/_ro/trn_rl_repo",
           "/root/.axon_site/_ro/pypackages"):
    if _p not in sys.path:
        sys.path.append(_p)

import numpy as np
import ml_dtypes
import concourse.bass as bass
import concourse.bacc as bacc
import concourse.mybir as mybir
import concourse.tile as tile
from concourse import bass_utils

N, IN_C, HID, OUT_C = 10000, 2048, 256, 256
NCORES = 8
ROWS = N // NCORES          # 1250
PROWS = 1280                # padded (10 x 128)
NT = PROWS // 128           # 10 row tiles per core
KT = IN_C // 128            # 16 k tiles
ST = NCORES * NT            # 80 source tiles in the padded gather space
NG = ST // 2                # 40 source super-tiles (DoubleRow pairs)
CH = [(0, 8), (8, 10)]   # AllGather chunk tile ranges

f32 = mybir.dt.float32
bf16 = mybir.dt.bfloat16
f8 = mybir.dt.float8e4
bfnp = ml_dtypes.bfloat16
f8np = ml_dtypes.float8_e4m3fn

# blob_a (bf16): W1 k-tiles then x row-tile-major k-tiles
OW1, OX = 0, KT * HID
BLOBA = KT * HID + NT * KT * 128
# blob_b (bf16): Wout k-tiles, bias (row 0), ones (row 0), identity
OWO, OB, OO, OI = 0, 6 * OUT_C, 6 * OUT_C + OUT_C, 6 * OUT_C + OUT_C + 128
BLOBB = OI + 128

# spmm processing order: super-tiles grouped by AllGather chunk
G_ORDER = [r * (NT // 2) + j for lo, hi in CH
           for r in range(NCORES) for j in range(lo // 2, hi // 2)]

LAST_EXEC_NS = None
LAST_RESULTS = None


def _install_trace_shim():
    try:
        import antenv.axon_hooks  # noqa: F401
        return
    except ImportError:
        pass
    try:
        import antenv
        from trn_agent_boot.trn_boot import _ntff_profile_via_ctypes
        hook = _ntff_profile_via_ctypes("/opt/axon/libaxon_pjrt.so")
        mod = types.ModuleType("antenv.axon_hooks")
        mod.get_axon_ntff_profile_hook = lambda: hook
        mod.set_axon_ntff_profile_hook = lambda h: None
        sys.modules["antenv.axon_hooks"] = mod
        antenv.axon_hooks = mod
    except Exception:
        pass


def _pack_adj(rows, cols, vals, core, scale):
    """fp8 dense A^T for this core's dest shard, laid out
    [128 src-part, NG super, 2 ktile, PROWS dest] (flattened free dim)."""
    lo, hi = core * ROWS, (core + 1) * ROWS
    m = (rows >= lo) & (rows < hi)
    r, c, v = rows[m] - lo, cols[m], vals[m] * scale
    A = np.zeros((NCORES * PROWS, PROWS), np.float32)
    src = (c // ROWS) * PROWS + (c % ROWS)
    np.add.at(A, (src, r), v)
    return np.ascontiguousarray(
        A.reshape(NG, 2, 128, PROWS).transpose(2, 0, 1, 3)
        .reshape(128, NG * 2 * PROWS)).astype(f8np)


def _build():
    nc = bacc.Bacc("TRN2", target_bir_lowering=False, debug=False,
                   num_devices=8)
    blob_a = nc.dram_tensor("blob_a", [128, BLOBA], bf16, kind="ExternalInput")
    blob_b = nc.dram_tensor("blob_b", [128, BLOBB], bf16, kind="ExternalInput")
    A_d = nc.dram_tensor("A_d", [128, 2 * NG * 2 * PROWS], f8,
                         kind="ExternalInput")
    out = nc.dram_tensor("out", [ROWS, OUT_C], f32, kind="ExternalOutput")

    DR = mybir.MatmulPerfMode.DoubleRow

    with tile.TileContext(nc) as tc:
        with tc.tile_pool(name="keep", bufs=1) as keep, \
             tc.tile_pool(name="dram", bufs=1, space="DRAM") as dram, \
             tc.tile_pool(name="pmm", bufs=2, space="PSUM") as pmm, \
             tc.tile_pool(name="pss", bufs=1, space="PSUM") as pss:

            h_sb0 = keep.tile([128, NT, HID], bf16)     # h0 node-major local
            ag_sb = keep.tile([128, NT, HID], f8)       # h0 fp8 (AG staging)
            h0a = keep.tile([128, ST, HID], f8)         # gathered global h0
            hT = keep.tile([128, 6, PROWS], bf16)       # feature-major concat
            wout_sb = keep.tile([128, BLOBB], bf16)
            pa_t = keep.tile([128, BLOBA], bf16)

            nc.sync.dma_start(wout_sb[:], blob_b[:])
            ident = wout_sb[:, OI:OI + 128]

            ag_ins, ag_outs = [], []
            for ci, (lo, hi) in enumerate(CH):
                w = hi - lo
                ag_ins.append(dram.tile([128, w * HID], f8,
                                        name=f"ag_in{ci}"))
                ag_outs.append(dram.tile([NCORES * 128, w * HID], f8,
                                         addr_space="Shared",
                                         name=f"ag_out{ci}"))

            # ---- phase A: h0 = x @ W1 (bf16), row-tile pipelined ----
            # Bulk streams (x, A) go through the Activation HWDGE queue;
            # latency-critical small DMAs stay on the SP (sync) queue.
            with nc.named_scope("h0_gemm"):
                nc.sync.dma_start(pa_t[:, OW1:OW1 + KT * HID],
                                  blob_a[:, OW1:OW1 + KT * HID])
                for t in range(NT):
                    o = OX + t * KT * 128
                    if t < 2:
                        nc.scalar.dma_start(pa_t[:, o:o + KT * 128],
                                            blob_a[:, o:o + KT * 128])
                for t in range(NT):
                    if t + 2 < NT:
                        o2 = OX + (t + 2) * KT * 128
                        nc.scalar.dma_start(pa_t[:, o2:o2 + KT * 128],
                                            blob_a[:, o2:o2 + KT * 128])
                    ps = pmm.tile([128, HID], f32, tag="mm")
                    o = OX + t * KT * 128
                    for k in range(KT):
                        nc.tensor.matmul(
                            ps[:],
                            pa_t[:, o + k * 128:o + (k + 1) * 128],
                            pa_t[:, OW1 + k * HID:OW1 + (k + 1) * HID],
                            start=(k == 0), stop=(k == KT - 1),
                        )
                    nc.vector.tensor_copy(h_sb0[:, t, :], ps[:])
                    nc.vector.tensor_copy(ag_sb[:, t, :], ps[:])
                    for half in range(2):
                        pt = pmm.tile([128, HID], f32, tag="mm")
                        ptb = pt[:].bitcast(bf16)[:, 0:128]
                        nc.tensor.transpose(
                            ptb, h_sb0[:, t, 128 * half:128 * (half + 1)],
                            ident)
                        nc.vector.tensor_copy(
                            hT[:, half, 128 * t:128 * (t + 1)], ptb)
                    for ci, (lo, hi) in enumerate(CH):
                        if t == hi - 1:
                            nc.sync.dma_start(
                                ag_ins[ci][:].rearrange(
                                    "p (a m) -> p a m", a=hi - lo),
                                ag_sb[:, lo:hi, :])

            # ---- phase B: AllGather h0 (fp8), three chunks, then unpack ----
            with nc.named_scope("allgather"):
                for ci, (lo, hi) in enumerate(CH):
                    nc.gpsimd.collective_compute(
                        "AllGather", mybir.AluOpType.bypass,
                        replica_groups=[list(range(NCORES))],
                        ins=[ag_ins[ci].opt()], outs=[ag_outs[ci].opt()],
                    )
                for ci, (lo, hi) in enumerate(CH):
                    for r in range(NCORES):
                        nc.sync.dma_start(
                            h0a[:, r * NT + lo:r * NT + hi, :],
                            ag_outs[ci][r * 128:(r + 1) * 128, :]
                            .rearrange("p (a m) -> p a m", a=hi - lo))

            # ---- phase C: SpMM, fp8 DoubleRow, A moving / h0 stationary ----
            # out[feat, dest] += sum_src h0a[src, feat] * A[src, dest]
            with nc.named_scope("spmm"):
                DCH = (512, 512, 256)
                for a in range(2):
                    ps6 = [[pss.tile([128, 512], f32, tag=f"s{fh}{d}",
                                     name=f"ps_s{fh}{d}")
                            for d in range(3)] for fh in range(2)]
                    for gi, g in enumerate(G_ORDER):
                        at = keep.tile([128, 2, PROWS], f8, tag="a", bufs=12)
                        off = (a * NG + g) * 2 * PROWS
                        nc.scalar.dma_start(
                            at[:], A_d[:, off:off + 2 * PROWS]
                            .rearrange("p (a m) -> p a m", a=2))
                        for fh in range(2):
                            lhs = h0a[:, 2 * g:2 * g + 2,
                                      128 * fh:128 * (fh + 1)]
                            dpos = 0
                            for d, w in enumerate(DCH):
                                nc.tensor.matmul(
                                    ps6[fh][d][:, 0:w], lhs,
                                    at[:, :, dpos:dpos + w],
                                    start=(gi == 0), stop=(gi == NG - 1),
                                    perf_mode=DR,
                                )
                                dpos += w
                    for fh in range(2):
                        dpos = 0
                        for d, w in enumerate(DCH):
                            nc.vector.tensor_copy(
                                hT[:, 2 + 2 * a + fh, dpos:dpos + w],
                                ps6[fh][d][:, 0:w])
                            dpos += w

            # ---- phase D: out = h @ Wout + b (bf16), split so only the
            # h1/h2 k-tiles sit in the post-SpMM tail; the h0 part lands in
            # DRAM during the AllGather gap and the tail accumulates on top.
            with nc.named_scope("out_gemm"):
                for t in range(NT):
                    ps = pmm.tile([128, OUT_C], f32, tag="mm")
                    nc.tensor.matmul(ps[:], wout_sb[0:1, OO:OO + 128],
                                     wout_sb[0:1, OB:OB + OUT_C],
                                     start=True, stop=False)
                    for k in range(2):
                        nc.tensor.matmul(
                            ps[:],
                            hT[:, k, 128 * t:128 * (t + 1)],
                            wout_sb[:, OWO + k * OUT_C:OWO + (k + 1) * OUT_C],
                            start=False, stop=(k == 1),
                        )
                    o_sb = keep.tile([128, OUT_C], f32, tag="osb", bufs=4)
                    nc.vector.tensor_copy(o_sb[:], ps[:])
                    rows = min(128, ROWS - 128 * t)
                    nc.sync.dma_start(out[128 * t:128 * t + rows, :],
                                      o_sb[:rows, :])
                for t in range(NT):
                    ps = pmm.tile([128, OUT_C], f32, tag="mm")
                    for k in range(2, 6):
                        nc.tensor.matmul(
                            ps[:],
                            hT[:, k, 128 * t:128 * (t + 1)],
                            wout_sb[:, OWO + k * OUT_C:OWO + (k + 1) * OUT_C],
                            start=(k == 2), stop=(k == 5),
                        )
                    o_sb = keep.tile([128, OUT_C], f32, tag="osb", bufs=4)
                    nc.vector.tensor_copy(o_sb[:], ps[:])
                    rows = min(128, ROWS - 128 * t)
                    nc.gpsimd.dma_start(out[128 * t:128 * t + rows, :],
                                        o_sb[:rows, :],
                                        accum_op=mybir.AluOpType.add)
    nc.compile()
    return nc


def kernel(x, adj1_rows, adj1_cols, adj1_vals, adj2_rows, adj2_cols, adj2_vals,
           W1, W_out, b_out):
    global LAST_EXEC_NS, LAST_RESULTS
    _install_trace_shim()
    x = np.asarray(x, np.float32)
    W1 = np.ascontiguousarray(np.asarray(W1, np.float32))
    W_out = np.ascontiguousarray(np.asarray(W_out, np.float32)).copy()
    b_out = np.asarray(b_out, np.float32).ravel()
    # undo the fp8-range pre-scaling of A1/A2 in the matching W_out rows
    W_out[HID:2 * HID] *= 1.0 / 16.0
    W_out[2 * HID:3 * HID] *= 1.0 / 32.0

    w1_cols = W1.reshape(KT, 128, HID).transpose(1, 0, 2).reshape(128, KT * HID)
    blob_b = np.zeros((128, BLOBB), np.float32)
    blob_b[:, OWO:OWO + 6 * OUT_C] = \
        W_out.reshape(6, 128, OUT_C).transpose(1, 0, 2).reshape(128, 6 * OUT_C)
    blob_b[0, OB:OB + OUT_C] = b_out
    blob_b[0, OO:OO + 128] = 1.0
    blob_b[:, OI:OI + 128] = np.eye(128, dtype=np.float32)
    blob_b = blob_b.astype(bfnp)

    a1r = np.asarray(adj1_rows, np.int64)
    a1c = np.asarray(adj1_cols, np.int64)
    a1v = np.asarray(adj1_vals, np.float32)
    a2r = np.asarray(adj2_rows, np.int64)
    a2c = np.asarray(adj2_cols, np.int64)
    a2v = np.asarray(adj2_vals, np.float32)

    in_maps = []
    for c in range(NCORES):
        xtp = np.zeros((IN_C, PROWS), np.float32)
        xtp[:, :ROWS] = x[c * ROWS:(c + 1) * ROWS].T
        blob_a = np.concatenate([
            w1_cols,
            xtp.reshape(KT, 128, NT, 128).transpose(1, 2, 0, 3)
            .reshape(128, NT * KT * 128),
        ], axis=1).astype(bfnp)
        A_pack = np.concatenate([
            _pack_adj(a1r, a1c, a1v, c, 16.0),
            _pack_adj(a2r, a2c, a2v, c, 32.0),
        ], axis=1)
        in_maps.append({"blob_a": blob_a, "blob_b": blob_b, "A_d": A_pack})

    nc = _build()
    try:
        res = bass_utils.run_bass_kernel_spmd(
            nc, in_maps, core_ids=list(range(NCORES)), trace=True,
            trace_cores=[0])
    except Exception:
        res = bass_utils.run_bass_kernel_spmd(
            nc, in_maps, core_ids=list(range(NCORES)), trace=False)
    LAST_EXEC_NS = res.exec_time_ns
    LAST_RESULTS = res
    return np.concatenate([res.results[c]["out"] for c in range(NCORES)], axis=0)
